# revision 6
# baseline (speedup 1.0000x reference)
"""Trainium2 Bass kernel for nn_Explainer segment_reduce (cdist + bidirectional
segment max/mean) on 8 NeuronCores.

Math (reference):
    ef_n = (h[ne0] + h[ne1])/2, ef_l = (h[le0] + h[le1])/2
    M = -cdist(ef_n, ef_l)                      # [En, El]
    out_n = seg_mean_rows(seg_max_cols(M))      # [Gn, Gl]
    out_l = seg_mean_cols(seg_max_rows(M))      # [Gn, Gl]
    out = (out_n + out_l)/2

Device computes psum = 2*u_n.u_l - |u_n|^2 - |u_l|^2 = -|u_n - u_l|^2 = -4*d^2
entirely in PSUM: two K=128 fp16 matmuls for the dot term plus one K=2 fp16
matmul adding the rank-1 terms -bl2[c] and -an2[r] (host-precomputed from the
quantized operands, so psum <= ~0 exactly). All segment reductions are then
plain MAX on psum values; host maps back via d = 0.5*sqrt(-v).

Sharding: core c owns node segments [8c, 8c+8) in per-segment lane bands
(segment s -> lanes [B_s, B_s+L_s), row-tiles t in [0, nrt)); dummy slots
duplicate the segment's first row (can't win a max, no masking needed).
Label columns replicated, each segment padded to a multiple of W=8 with
duplicate edges.

Per tile: PE fills psum groups; ACT (+ GpSimd for some groups) converts fp32
psum -> fp16 strip (pure copy); DVE does the W-block max tree (row side) and
the running col max across tiles (col side, some groups on GpSimd).
Host: fold blocks per label segment + sqrt + masked means; band-collapse the
col accumulator + sqrt + masked means; assemble [64, 64].
"""
import numpy as np

import concourse.bacc as bacc
import concourse.tile as tile
import concourse.mybir as mybir
from concourse.alu_op_type import AluOpType
from concourse.bass_utils import run_bass_kernel_spmd

P = 128
N_CORES = 8
GN = GL = 64
D = 256
W = 8                      # label block width for the row-side max tree
F16 = mybir.dt.float16
F32 = mybir.dt.float32

_prog_cache = {}


def _groups(C):
    """psum column groups: [128,1024] tiles (2 banks) + optional 512 rem."""
    gs = [(i * 1024, 1024) for i in range(C // 1024)]
    if C % 1024:
        gs.append((C - 512, 512))
    return gs


def _build(nrt: int, C: int, gp_conv=(), gp_colmax=()):
    B = C // W
    groups = _groups(C)

    nc = bacc.Bacc("TRN2", target_bir_lowering=False, debug=False,
                   num_devices=N_CORES)
    ulT_in = nc.dram_tensor("ulT", [P, 2 * C], F16, kind="ExternalInput")
    unT_in = nc.dram_tensor("unT", [P, nrt * 2 * P], F16, kind="ExternalInput")
    rhs2_in = nc.dram_tensor("rhs2", [2, C], F16, kind="ExternalInput")
    an2_in = nc.dram_tensor("an2", [1, nrt * P], F16, kind="ExternalInput")
    rowout = nc.dram_tensor("rowout", [P, nrt * B], F16, kind="ExternalOutput")
    collout = nc.dram_tensor("collout", [P, C], F16, kind="ExternalOutput")

    with tile.TileContext(nc) as tc:
        with (
            tc.tile_pool(name="persist", bufs=1) as pp,
            tc.tile_pool(name="strip", bufs=2) as sp,
            tc.tile_pool(name="s1", bufs=2) as s1p,
            tc.tile_pool(name="row", bufs=3) as rp,
        ):
            u_lT = pp.tile([P, 2, C], F16, tag="u_lT")
            u_nT = pp.tile([P, nrt, 2, P], F16, tag="u_nT")
            rhs2 = pp.tile([2, C], F16, tag="rhs2")
            lhsT2 = pp.tile([2, nrt * P], F16, tag="lhsT2")
            coll = pp.tile([P, C], F16, tag="coll")

            nc.gpsimd.memset(lhsT2[0:1, :], 1.0)
            nc.sync.dma_start(lhsT2[1:2, :], an2_in[:])
            nc.sync.dma_start(u_nT[:].rearrange("p t k q -> p (t k q)"),
                              unT_in[:])
            nc.sync.dma_start(rhs2[:], rhs2_in[:])
            for g0, w in groups:
                for k in range(2):
                    nc.sync.dma_start(u_lT[:, k, g0:g0 + w],
                                      ulT_in[:, k * C + g0:k * C + g0 + w])

            collv = coll[:].rearrange("p (b w) -> p b w", w=W)
            with (
                tc.tile_pool(name="ps1024", bufs=3, space="PSUM") as pg,
                tc.tile_pool(name="ps512", bufs=2, space="PSUM") as pr,
            ):
                for t in range(nrt):
                    strip = sp.tile([P, B, W], F16, tag="strip")
                    stripf = strip[:].rearrange("p b w -> p (b w)")
                    for gi, (g0, w) in enumerate(groups):
                        pool = pg if w == 1024 else pr
                        pt = pool.tile([P, w], F32, tag=f"dot{w}")
                        nchunk = w // 512
                        for k in range(2):
                            for j in range(nchunk):
                                osl = slice(j * 512, (j + 1) * 512)
                                csl = slice(g0 + j * 512, g0 + (j + 1) * 512)
                                nc.tensor.matmul(pt[:, osl], u_nT[:, t, k, :],
                                                 u_lT[:, k, csl],
                                                 start=(k == 0), stop=False)
                        for j in range(nchunk):
                            osl = slice(j * 512, (j + 1) * 512)
                            csl = slice(g0 + j * 512, g0 + (j + 1) * 512)
                            nc.tensor.matmul(pt[:, osl],
                                             lhsT2[:, t * P:(t + 1) * P],
                                             rhs2[:, csl],
                                             start=False, stop=True)
                        dst = coll[:, g0:g0 + w] if t == 0 else stripf[:, g0:g0 + w]
                        if gi in gp_conv:
                            nc.gpsimd.tensor_copy(dst, pt[:])
                        else:
                            nc.scalar.copy(dst, pt[:])
                    # col side: running max into coll (t=0 wrote coll directly)
                    if t > 0:
                        for gi, (g0, w) in enumerate(groups):
                            eng = nc.gpsimd if gi in gp_colmax else nc.vector
                            eng.tensor_max(coll[:, g0:g0 + w],
                                           coll[:, g0:g0 + w],
                                           stripf[:, g0:g0 + w])
                            if t == nrt - 1:
                                nc.sync.dma_start(collout[:, g0:g0 + w],
                                                  coll[:, g0:g0 + w])
                    # row side: W-block max tree -> [P, B]
                    src = collv if t == 0 else strip[:]
                    s1 = s1p.tile([P, B, 4], F16, tag="s1")
                    nc.vector.tensor_max(s1[:], src[:, :, 0:4], src[:, :, 4:8])
                    nc.vector.tensor_max(s1[:, :, 0:2], s1[:, :, 0:2],
                                         s1[:, :, 2:4])
                    rst = rp.tile([P, B], F16, tag="rst")
                    nc.vector.tensor_max(rst[:], s1[:, :, 0], s1[:, :, 1])
                    nc.sync.dma_start(rowout[:, t * B:(t + 1) * B], rst[:])

    nc.compile()
    return nc


def _get_program(nrt, C):
    key = (nrt, C)
    if key not in _prog_cache:
        _prog_cache[key] = _build(nrt, C)
    return _prog_cache[key]


def _band_layout(sizes, nrt):
    """Lane bands: segment s gets L_s = ceil(size_s/nrt) lanes."""
    L = [-(-int(s) // nrt) if s > 0 else 0 for s in sizes]
    B = np.concatenate([[0], np.cumsum(L)]).astype(np.int64)
    return B, L


def kernel(h, node_edge, node_batch, label_edge, label_batch):
    h = np.asarray(h)
    ne = np.asarray(node_edge).astype(np.int64)
    nb = np.asarray(node_batch).astype(np.int64)
    le = np.asarray(label_edge).astype(np.int64)
    lb = np.asarray(label_batch).astype(np.int64)

    cn = np.bincount(nb, minlength=GN).astype(np.int64)
    cl = np.bincount(lb, minlength=GL).astype(np.int64)
    nb_off = np.concatenate([[0], np.cumsum(cn)])
    lb_off = np.concatenate([[0], np.cumsum(cl)])

    # ---- label columns: each segment padded to a multiple of W with
    # duplicate edges; then global pad to a multiple of 512 with col 0 dups
    bg = -(-cl // W)                       # blocks per segment
    b_off = np.concatenate([[0], np.cumsum(bg)])
    B_real = int(b_off[-1])
    C_real = B_real * W
    C = -(-C_real // 512) * 512
    B = C // W

    col_edge = np.zeros(C, np.int64)
    for g in range(GL):
        n_g = int(cl[g])
        if n_g == 0:
            continue
        width = int(bg[g]) * W
        k = np.arange(width)
        col_edge[b_off[g] * W + k] = lb_off[g] + k % n_g

    hf = h.astype(np.float32)
    u_l = hf[le[0][col_edge]] + hf[le[1][col_edge]]            # [C, 256] fp32
    b16 = u_l.astype(np.float16)                               # quantized b
    bl2 = (b16.astype(np.float32) ** 2).sum(axis=1)            # |b|^2
    ulT = np.ascontiguousarray(
        b16.T.reshape(2, P, C).transpose(1, 0, 2).reshape(P, 2 * C))
    rhs2 = np.ascontiguousarray(
        np.stack([-bl2, -np.ones(C, np.float32)]).astype(np.float16))

    # ---- node rows: per-core lane bands over 8 segments; dummy slots
    # duplicate the segment's first row
    core_sizes = cn.reshape(N_CORES, 8)
    nrt = max(1, int(-(-core_sizes.sum(1).max() // P)))
    while max(sum(-(-int(s) // nrt) for s in core_sizes[c] if s > 0)
              for c in range(N_CORES)) > P:
        nrt += 1
    nrows = nrt * P

    in_maps = []
    band_info = []
    for c in range(N_CORES):
        Bo, L = _band_layout(core_sizes[c], nrt)
        assert Bo[-1] <= P
        slot = np.zeros(nrows, np.int64)
        # fallback row for fully-unused lanes (any valid index)
        slot[:] = min(int(nb_off[8 * c]), ne.shape[1] - 1)
        for s in range(8):
            g = 8 * c + s
            n_g = int(cn[g])
            if n_g == 0:
                continue
            lanes_all = np.arange(Bo[s], Bo[s + 1])
            for tt in range(nrt):
                slot[tt * P + lanes_all] = nb_off[g]   # seg dup default
            j = np.arange(n_g)
            lanes = Bo[s] + j // nrt
            ts = j % nrt
            slot[ts * P + lanes] = nb_off[g] + j
        u_n = hf[ne[0][slot]] + hf[ne[1][slot]]                 # [nrows, 256]
        a16 = (2.0 * u_n).astype(np.float16)                    # quantized a
        an2 = ((a16.astype(np.float32) ** 2).sum(axis=1) * 0.25)
        # unT layout: [p(K%128), t, k, row] ; row r of tile t = a16[t*P + r]
        a = a16.reshape(nrt, P, 2, P)        # [t, row, k, p]
        unT = np.ascontiguousarray(a.transpose(3, 0, 2, 1).reshape(P, -1))
        an2_16 = np.ascontiguousarray(
            an2.astype(np.float16).reshape(nrt, P).reshape(1, -1))
        in_maps.append({
            "ulT": ulT,
            "unT": unT,
            "rhs2": rhs2,
            "an2": an2_16,
        })
        band_info.append((Bo, L))

    nc = _get_program(nrt, C)
    res = run_bass_kernel_spmd(nc, in_maps, core_ids=list(range(N_CORES)))

    # ---- host unpack -----------------------------------------------------
    out_n = np.zeros((GN, GL), np.float64)
    out_l = np.zeros((GN, GL), np.float64)
    # per-column weights/segments for the col side
    col_w = np.zeros(C, np.float64)
    for g in range(GL):
        col_w[b_off[g] * W:b_off[g] * W + int(cl[g])] = 1.0
    seg_bounds = (b_off[:-1], b_off[1:])
    for c in range(N_CORES):
        r = res.results[c]
        rowe = r["rowout"].astype(np.float64).reshape(P, nrt, B)
        colle = r["collout"].astype(np.float64)                 # [128, C]
        Bo, L = band_info[c]
        for s in range(8):
            g = 8 * c + s
            n_g = int(cn[g])
            if n_g == 0:
                continue
            j = np.arange(n_g)
            lanes = Bo[s] + j // nrt
            ts = j % nrt
            blk = rowe[lanes, ts, :]                            # [n_g, B]
            # fold blocks per label segment: max, then d = 0.5*sqrt(-v)
            segmax = np.maximum.reduceat(
                blk[:, :B_real], b_off[:-1].clip(0, B_real - 1), axis=1)
            d = 0.5 * np.sqrt(np.maximum(-segmax, 0.0))
            row_mean = -d.mean(axis=0)
            row_mean[cl == 0] = 0.0
            out_n[g] = row_mean

            ecol = colle[Bo[s]:Bo[s] + L[s], :].max(axis=0)     # [C]
            dcol = 0.5 * np.sqrt(np.maximum(-ecol, 0.0))
            sums = np.add.reduceat(
                (dcol * col_w)[:B_real * W].reshape(-1, W).reshape(B_real * W),
                (b_off[:-1] * W).clip(0, B_real * W - 1))
            col_mean = -(sums / np.maximum(cl, 1))
            col_mean[cl == 0] = 0.0
            out_l[g] = col_mean

    return ((out_n + out_l) * 0.5).astype(np.float32)
